# revision 33
# baseline (speedup 1.0000x reference)
"""GCN encoder (3-layer, N=10000, E=160000, d=512) on 8 Trainium2 NeuronCores.

Sharding: nodes destination-sharded 1250/core. Latency-oriented schedule
(measured ~375us vs the 823us GEMM->AllGather->scatter baseline):

- The per-layer serial chain is collapsed: each window's GEMM for layer
  l+1 is interleaved into layer l's scatter phase (right after that
  window's aggregation is evicted), so the AllGather for layer l+1 fires
  mid-scatter and hides behind the remaining gather work.
- The AllGather is split in two half-shard collectives (windows 0-4 and
  5-9 of every rank). Gather chunks are split by source half so a
  layer's first chunks depend only on the first (early-fired)
  collective; by the time half-1 chunks issue, the second has landed.
- The gathered table is fp8e3 (e3m4) with per-layer scales SCALES
  (range-checked against the input distribution; 1/S folded into the
  PSUM-eviction scale) - halves gather/collective bytes at full PE rate;
  one-hot stationaries are fp8e3 too (small ints, exact). rel err
  ~0.005 vs the 2e-2 gate.
- Window PAIRS (2w, 2w+1) share one deduped gather set (cuts SWDGE
  instruction count ~25%: the per-chunk ~1us Pool-engine descriptor-gen
  fixed cost is the bottleneck), and each gathered row PACKs 2 adjacent
  nodes (the pair view is a pure reinterpretation of the node table, so
  the collective layout is unchanged). Each chunk feeds 2 windows x 2
  lanes = 4 one-hot matmuls. Supergroups of <=2 pairs bound live PSUM
  accumulators (4 scat + 2 gemm + 1 transpose banks < 8).
- Aggregation stays in the proven form: one indirect DMA per 128-row
  chunk (TRN2 SWDGE honors exactly one dynamic offset per output
  partition row - multi-offset APs replicate idx[p,0], probe-verified;
  the batched InstDMAGatherAnt path wedges this terminal's cores, also
  probe-verified), scatter-add via one-hot stationary matmuls with fp32
  PSUM accumulation, self-loops via identity-stationary matmuls from
  the SBUF-resident fp8 shard, relu + symmetric-norm post-scale fused
  into the PSUM eviction. One-hots are streamed from DRAM per
  GRP-chunk group (too large for SBUF residency).
"""

import sys

sys.path.insert(0, "/opt/trn_rl_repo")

import numpy as np
import ml_dtypes

import concourse.bacc as bacc
import concourse.bass as bass
import concourse.mybir as mybir
from concourse import tile
from concourse.bass_utils import run_bass_kernel_spmd

BF16 = ml_dtypes.bfloat16
F8E3 = ml_dtypes.float8_e3m4
SCALES = (8.0, 16.0, 64.0)  # per-layer fp8 table scales (range-validated on inputs)

N = 10000
F = 512
NCORES = 8
P = N // NCORES          # 1250 nodes per core
NW = 10                  # dest windows per core (128 rows each)
PW = NW * 128            # 1280 padded nodes per core
HR = 640                 # rows per table half (windows 0-4 / 5-9)
PACK = 2                 # nodes packed per gathered table row
NPAIR = 5                # window pairs (2w, 2w+1) sharing one gather set
SG = ((0, 1), (2, 3), (4,))  # supergroups of pairs (PSUM: 2 pairs = 4 banks)
GRP = 8                  # chunks per gather/one-hot staging tile


def _preprocess(x, edge_index, Ws, bs):
    """Host-side: graph normalization + per-core gather/scatter plans."""
    ei = np.asarray(edge_index).astype(np.int64)
    deg = np.ones(N, np.float32)
    np.add.at(deg, ei[1], 1.0)
    dis = np.where(deg > 0, 1.0 / np.sqrt(np.maximum(deg, 1.0)), 0.0).astype(
        np.float32
    )
    row, col = ei[0], ei[1]

    # bucket edges by (core, window-pair, source-half); keep per-edge info
    edges = [[[None, None] for _ in range(NPAIR)] for _ in range(NCORES)]
    core_of = col // P
    wloc = (col - core_of * P) // 128
    half_of = (row % P) // HR  # 0 for local rows 0..639, 1 for 640..1249
    for c in range(NCORES):
        mc = core_of == c
        rc, cc, hc = row[mc], col[mc] - c * P, half_of[mc]
        wc = cc // 128
        for pi in range(NPAIR):
            for h in range(2):
                m = (wc // 2 == pi) & (hc == h)
                edges[c][pi][h] = (rc[m], cc[m] % 128, wc[m] % 2)

    def packrows(c, pi, h):
        s = edges[c][pi][h][0]
        lam = (s % P) - h * HR
        return (s // P) * (HR // PACK) + lam // PACK, (s % P) % PACK

    # uniform chunk counts over deduped PACKed rows (SPMD: one program)
    nch = [
        [
            max(
                1,
                max(
                    (len(np.unique(packrows(c, pi, h)[0])) + 127) // 128
                    for c in range(NCORES)
                ),
            )
            for h in range(2)
        ]
        for pi in range(NPAIR)
    ]

    # chunk processing order: per supergroup: all half-0 chunks (pair
    # ascending), then per-pair half-1 chunks. chunk_map rows:
    #   (pair, is_first_of_pair, is_last_of_pair)
    chunk_map = []
    chunk_halves = []
    order = []  # (pi, h, k)
    for sg in SG:
        for pi in sg:
            for k in range(nch[pi][0]):
                order.append((pi, 0, k))
                chunk_map.append((pi, k == 0, False))
                chunk_halves.append(0)
        for pi in sg:
            for k in range(nch[pi][1]):
                order.append((pi, 1, k))
                chunk_map.append((pi, False, k == nch[pi][1] - 1))
                chunk_halves.append(1)
    nchunk = len(order)
    chunk_base = {}
    b = 0
    for (pi, h, k) in order:
        chunk_base[(pi, h, k)] = b
        b += 1

    per_core = []
    for c in range(NCORES):
        idx32 = np.zeros((128, nchunk), np.int32)
        onehot = np.zeros((128, nchunk * 2 * PACK * 128), np.float32)
        for pi in range(NPAIR):
            for h in range(2):
                _, d, v = edges[c][pi][h]
                prow, lane = packrows(c, pi, h)
                uniqp, invp = np.unique(prow, return_inverse=True)
                for k in range(nch[pi][h]):
                    t = chunk_base[(pi, h, k)]
                    seg = uniqp[k * 128 : (k + 1) * 128]
                    idx32[: len(seg), t] = seg
                # edge e: pack slot sigma, lane l, window parity v, dest d:
                # one-hot block = ((chunk*2 + v)*PACK + l)
                sigma_e = invp
                t0 = chunk_base[(pi, h, 0)]
                blk = ((t0 + sigma_e // 128) * 2 + v) * PACK + lane
                np.add.at(
                    onehot,
                    (sigma_e % 128, blk * 128 + d),
                    np.float32(1.0),
                )
        onehot = onehot.astype(F8E3)

        dis_sc = np.zeros((128, 6 * NW), np.float32)
        nloc = np.arange(P)
        dv = dis[c * P : (c + 1) * P]
        for l in range(3):
            dis_sc[nloc % 128, l * NW + nloc // 128] = SCALES[l] * dv
            dis_sc[nloc % 128, (3 + l) * NW + nloc // 128] = dv / SCALES[l]

        xT = np.zeros((F, PW), BF16)
        xT[:, :P] = np.asarray(x)[c * P : (c + 1) * P].T.astype(BF16)
        per_core.append(
            {"xt": xT, "dis": dis_sc, "gidx": idx32, "onehot": onehot}
        )

    wall = np.stack([np.asarray(w).astype(BF16) for w in Ws])
    ident = np.eye(128, dtype=BF16)
    ident8 = np.eye(128, dtype=F8E3)
    has_bias = any(np.any(np.asarray(b)) for b in bs)
    shared = {"wall": wall, "ident": ident, "ident8": ident8}
    if has_bias:
        brep = np.stack([np.asarray(b).astype(BF16) for b in bs])
        shared["brep"] = brep.reshape(1, 3 * F)
        for c in range(NCORES):
            iv = np.zeros((1, PW), BF16)
            iv[0, :P] = (1.0 / dis[c * P : (c + 1) * P]).astype(BF16)
            per_core[c]["invdis"] = iv
    meta = {
        "nchunk": nchunk,
        "chunk_map": tuple(chunk_map),
        "chunk_halves": tuple(chunk_halves),
        "has_bias": has_bias,
    }
    return per_core, shared, meta


def _build(meta, mock_cc=False, repeat=1):
    nchunk = meta["nchunk"]
    chunk_map, has_bias = meta["chunk_map"], meta["has_bias"]
    chunk_halves = meta["chunk_halves"]
    bf = mybir.dt.bfloat16
    f8 = mybir.dt.float8e3
    f32 = mybir.dt.float32

    nc = bacc.Bacc(
        "TRN2",
        target_bir_lowering=False,
        debug=False,
        num_devices=1 if mock_cc else NCORES,
        dynamic_dma_scratch_size=65536,
    )
    xt_d = nc.dram_tensor("xt", [F, PW], bf, kind="ExternalInput")
    wall_d = nc.dram_tensor("wall", [3, F, F], bf, kind="ExternalInput")
    dis_d = nc.dram_tensor("dis", [128, 6 * NW], f32, kind="ExternalInput")
    gidx_d = nc.dram_tensor(
        "gidx", [128, nchunk], mybir.dt.int32, kind="ExternalInput"
    )
    oh_d = nc.dram_tensor(
        "onehot", [128, nchunk * 2 * PACK * 128], f8, kind="ExternalInput"
    )
    id_d = nc.dram_tensor("ident", [128, 128], bf, kind="ExternalInput")
    id8_d = nc.dram_tensor("ident8", [128, 128], f8, kind="ExternalInput")
    if has_bias:
        brep_d = nc.dram_tensor("brep", [1, 3 * F], bf, kind="ExternalInput")
        invdis_d = nc.dram_tensor("invdis", [1, PW], bf, kind="ExternalInput")
    y_d = nc.dram_tensor("y", [PW, F], f32, kind="ExternalOutput")

    with tile.TileContext(nc) as tc:
        with (
            tc.tile_pool(name="const", bufs=1) as cp,
            tc.tile_pool(name="work", bufs=1) as wp,
            tc.tile_pool(name="gatp", bufs=3) as gp,
            tc.tile_pool(name="ohp", bufs=3) as op,
            tc.tile_pool(name="psum", bufs=1, space="PSUM") as pp,
            tc.tile_pool(name="ptr", bufs=2, space="PSUM") as ppt,
            tc.tile_pool(name="ccin_p", bufs=2, space="DRAM") as dp_in,
            tc.tile_pool(name="ccout_p", bufs=2, space="DRAM") as dp_out,
            tc.tile_pool(name="tok_p", bufs=1, space="DRAM") as dp_tok,
        ):
            # constants
            w_t = cp.tile([128, 3 * 4 * F], bf, name="w_t")
            for l in range(3):
                for kc in range(4):
                    nc.sync.dma_start(
                        w_t[:, (l * 4 + kc) * F : (l * 4 + kc + 1) * F],
                        wall_d[l, kc * 128 : (kc + 1) * 128, :],
                    )
            dis_t = cp.tile([128, 6 * NW], f32, name="dis_t")
            nc.sync.dma_start(dis_t[:], dis_d[:])
            idx_t = cp.tile([128, nchunk], mybir.dt.int32, name="idx_t")
            nc.sync.dma_start(idx_t[:], gidx_d[:])
            id_t = cp.tile([128, 128], bf, name="id_t")
            nc.sync.dma_start(id_t[:], id_d[:])
            id8_t = cp.tile([128, 128], f8, name="id8_t")
            nc.sync.dma_start(id8_t[:], id8_d[:])
            if has_bias:
                brep_t = cp.tile([1, 3 * F], bf, name="brep_t")
                nc.sync.dma_start(brep_t[:], brep_d[:])
                invdis_t = cp.tile([1, PW], bf, name="invdis_t")
                nc.sync.dma_start(invdis_t[:], invdis_d[:])

            def allgather(cc_in, half, l, rep):
                """AG of cc_in rows [half*HR:(half+1)*HR) of every rank into a
                [NCORES*HR, F] table (per-rank stripes of HR rows)."""
                cc_out = dp_out.tile(
                    [NCORES * HR, F],
                    f8,
                    tag=f"ccoh{half}",
                    addr_space="Local" if mock_cc else "Shared",
                    name=f"ccout{l}_{half}_{rep}",
                )
                src = cc_in[half * HR : (half + 1) * HR, :]
                if mock_cc:
                    for r in range(NCORES):
                        nc.sync.dma_start(cc_out[r * HR : (r + 1) * HR, :], src)
                else:
                    nc.gpsimd.collective_compute(
                        "AllGather",
                        mybir.AluOpType.bypass,
                        replica_groups=[list(range(NCORES))],
                        ins=[src],
                        outs=[cc_out[:]],
                    )
                return cc_out

            def gemm_window(l, w, zt_src, h_t, cc_in, rep):
                """H'[w] = dis * (Z[w] @ W_l) -> h_t[:, w] (bf16) -> cc_in."""
                pg = pp.tile(
                    [128, F], f32, tag="gemm", bufs=2, name=f"pg{l}_{w}_{rep}"
                )
                for kc in range(4):
                    nc.tensor.matmul(
                        pg[:],
                        zt_src[:, kc * PW + w * 128 : kc * PW + (w + 1) * 128],
                        w_t[:, (l * 4 + kc) * F : (l * 4 + kc + 1) * F],
                        start=(kc == 0),
                        stop=(kc == 3),
                    )
                nc.scalar.activation(
                    h_t[:, w * F : (w + 1) * F],
                    pg[:],
                    mybir.ActivationFunctionType.Copy,
                    scale=dis_t[:, l * NW + w : l * NW + w + 1],
                )
                nc.sync.dma_start(
                    cc_in[w * 128 : (w + 1) * 128, :],
                    h_t[:, w * F : (w + 1) * F],
                )

            tok_d = None
            for rep in range(repeat):
                # ---- prologue: load Z^T, GEMM layer 0, fire AGs ----
                zt = wp.tile([128, 4 * PW], bf, tag="ZT", name=f"zt0_{rep}")
                for kc in range(4):
                    nc.sync.dma_start(
                        zt[:, kc * PW : (kc + 1) * PW],
                        xt_d[kc * 128 : (kc + 1) * 128, :],
                    )
                if rep > 0:
                    # timing builds: chain rep on previous rep's output so the
                    # marginal-slope measurement reflects single-shot latency
                    # (corrupts 16 staged Z values; repeat>1 is never used for
                    # the graded numerics)
                    tok_s = wp.tile([1, 16], f32, tag="toks", name=f"toks{rep}")
                    nc.sync.dma_start(tok_s[:], tok_d[:])
                    nc.scalar.activation(
                        zt[0:1, 0:16],
                        tok_s[:],
                        mybir.ActivationFunctionType.Copy,
                    )
                h_t = wp.tile([128, NW * F], f8, tag="H", bufs=2, name=f"h0_{rep}")
                cc_in = dp_in.tile([PW, F], f8, tag="ccin", name=f"ccin0_{rep}")
                cc_out_h = [None, None]
                for w in range(NW):
                    gemm_window(0, w, zt, h_t, cc_in, rep)
                    if w == 4:
                        cc_out_h[0] = allgather(cc_in, 0, 0, rep)
                cc_out_h[1] = allgather(cc_in, 1, 0, rep)

                # ---- 3 scatter phases ----
                for l in range(3):
                    z_t = (
                        wp.tile([128, NW * F], bf, tag="Z", name=f"z{l}_{rep}")
                        if l < 2
                        else None
                    )
                    ztn = (
                        wp.tile([128, 4 * PW], bf, tag="ZT", name=f"zt{l + 1}_{rep}")
                        if l < 2
                        else None
                    )
                    h_next = (
                        wp.tile(
                            [128, NW * F], f8, tag="H", bufs=2, name=f"h{l + 1}_{rep}"
                        )
                        if l < 2
                        else None
                    )
                    cc_in_next = (
                        dp_in.tile([PW, F], f8, tag="ccin", name=f"ccin{l + 1}_{rep}")
                        if l < 2
                        else None
                    )
                    cc_next = [None, None]

                    ps_w = {}
                    gat = None
                    ohg = None
                    for t in range(nchunk):
                        if t % GRP == 0:
                            gat = gp.tile(
                                [128, GRP * PACK * F],
                                f8,
                                tag="gat",
                                name=f"gat{l}_{t // GRP}_{rep}",
                            )
                            ohg = op.tile(
                                [128, GRP * 2 * PACK * 128],
                                f8,
                                tag="oh",
                                name=f"oh{l}_{t // GRP}_{rep}",
                            )
                            ghi = min(t + GRP, nchunk)
                            nc.sync.dma_start(
                                ohg[:, : (ghi - t) * 2 * PACK * 128],
                                oh_d[:, t * 2 * PACK * 128 : ghi * 2 * PACK * 128],
                            )
                        c = t % GRP
                        pi, first, last = chunk_map[t]
                        nc.gpsimd.indirect_dma_start(
                            gat[:, c * PACK * F : (c + 1) * PACK * F],
                            None,
                            cc_out_h[chunk_halves[t]][:].rearrange(
                                "(r q) f -> r (q f)", q=PACK
                            ),
                            bass.IndirectOffsetOnAxis(
                                ap=idx_t[:, t : t + 1], axis=0
                            ),
                        )
                        if first:
                            for v in range(2):
                                w = 2 * pi + v
                                ps = pp.tile(
                                    [128, F],
                                    f32,
                                    tag=f"scat{w % 4}",
                                    name=f"ps{l}_{w}_{rep}",
                                )
                                ps_w[w] = ps
                                # self-loop: psum[d] += H'[d]
                                nc.tensor.matmul(
                                    ps[:],
                                    id8_t[:],
                                    h_t[:, w * F : (w + 1) * F],
                                    start=True,
                                    stop=False,
                                )
                        for v in range(2):
                            ps = ps_w[2 * pi + v]
                            for ln in range(PACK):
                                nc.tensor.matmul(
                                    ps[:],
                                    ohg[
                                        :,
                                        ((c * 2 + v) * PACK + ln)
                                        * 128 : ((c * 2 + v) * PACK + ln + 1)
                                        * 128,
                                    ],
                                    gat[
                                        :,
                                        (c * PACK + ln) * F : (c * PACK + ln + 1) * F,
                                    ],
                                    start=False,
                                    stop=(
                                        last and ln == PACK - 1 and not has_bias
                                    ),
                                )
                        if last:
                          for v in range(2):
                            w = 2 * pi + v
                            ps = ps_w[w]
                            if has_bias:
                                nc.tensor.matmul(
                                    ps[:],
                                    invdis_t[0:1, w * 128 : (w + 1) * 128],
                                    brep_t[0:1, l * F : (l + 1) * F],
                                    start=False,
                                    stop=True,
                                )
                            if l < 2:
                                nc.scalar.activation(
                                    z_t[:, w * F : (w + 1) * F],
                                    ps[:],
                                    mybir.ActivationFunctionType.Relu,
                                    scale=dis_t[:, (3 + l) * NW + w : (3 + l) * NW + w + 1],
                                )
                                for fc in range(4):
                                    pt = ppt.tile(
                                        [128, 128],
                                        bf,
                                        tag="tr",
                                        name=f"pt{l}_{w}_{fc}_{rep}",
                                    )
                                    nc.tensor.transpose(
                                        pt[:],
                                        z_t[
                                            :,
                                            w * F + fc * 128 : w * F + (fc + 1) * 128,
                                        ],
                                        id_t[:],
                                    )
                                    nc.vector.tensor_copy(
                                        ztn[
                                            :,
                                            fc * PW + w * 128 : fc * PW + (w + 1) * 128,
                                        ],
                                        pt[:],
                                    )
                                gemm_window(l + 1, w, ztn, h_next, cc_in_next, rep)
                                if w == 4:
                                    cc_next[0] = allgather(cc_in_next, 0, l + 1, rep)
                                elif w == 9:
                                    cc_next[1] = allgather(cc_in_next, 1, l + 1, rep)
                            else:
                                yw = wp.tile(
                                    [128, F], f32, tag="Y", name=f"yw{w}_{rep}"
                                )
                                nc.scalar.activation(
                                    yw[:],
                                    ps[:],
                                    mybir.ActivationFunctionType.Copy,
                                    scale=dis_t[:, (3 + l) * NW + w : (3 + l) * NW + w + 1],
                                )
                                nc.sync.dma_start(
                                    y_d[w * 128 : (w + 1) * 128, :], yw[:]
                                )
                                if w == 9 and repeat > 1:
                                    tok_d = dp_tok.tile(
                                        [1, 16], f32, tag="tok", name=f"tok{rep}"
                                    )
                                    nc.sync.dma_start(tok_d[:], yw[0:1, 0:16])
                    if l < 2:
                        zt = ztn
                        h_t = h_next
                        cc_in = cc_in_next
                        cc_out_h = cc_next

    nc.compile()
    return nc


_CACHE = {}


def _get_program(meta):
    key = (meta["nchunk"], meta["chunk_map"], meta["has_bias"], meta["chunk_halves"])
    if key not in _CACHE:
        _CACHE[key] = _build(meta)
    return _CACHE[key]


def kernel(x, edge_index, W1, b1, W2, b2, W3, b3):
    per_core, shared, meta = _preprocess(
        x, edge_index, [W1, W2, W3], [b1, b2, b3]
    )
    nc = _get_program(meta)
    in_maps = [dict(pc, **shared) for pc in per_core]
    res = run_bass_kernel_spmd(nc, in_maps, list(range(NCORES)))
    out = np.concatenate(
        [res.results[c]["y"][:P] for c in range(NCORES)], axis=0
    )
    return out.astype(np.float32)


# revision 35
# speedup vs baseline: 1.4060x; 1.4060x over previous
"""GCN encoder (3-layer, N=10000, E=160000, d=512) on 8 Trainium2 NeuronCores.

Sharding: nodes destination-sharded 1250/core. Latency-oriented schedule
(measured ~375us vs the 823us GEMM->AllGather->scatter baseline):

- The per-layer serial chain is collapsed: each window's GEMM for layer
  l+1 is interleaved into layer l's scatter phase (right after that
  window's aggregation is evicted), so the AllGather for layer l+1 fires
  mid-scatter and hides behind the remaining gather work.
- The AllGather is split in two half-shard collectives (windows 0-4 and
  5-9 of every rank). Gather chunks are split by source half so a
  layer's first chunks depend only on the first (early-fired)
  collective; by the time half-1 chunks issue, the second has landed.
- The gathered table is fp8e3 (e3m4) with per-layer scales SCALES
  (range-checked against the input distribution; 1/S folded into the
  PSUM-eviction scale) - halves gather/collective bytes at full PE rate;
  one-hot stationaries are fp8e3 too (small ints, exact). rel err
  ~0.005 vs the 2e-2 gate.
- Window PAIRS (2w, 2w+1) share one deduped gather set (cuts SWDGE
  instruction count ~25%: the per-chunk ~1us Pool-engine descriptor-gen
  fixed cost is the bottleneck), and each gathered row PACKs 2 adjacent
  nodes (the pair view is a pure reinterpretation of the node table, so
  the collective layout is unchanged). Each chunk feeds 2 windows x 2
  lanes = 4 one-hot matmuls. Supergroups of <=2 pairs bound live PSUM
  accumulators (4 scat + 2 gemm + 1 transpose banks < 8).
- Aggregation stays in the proven form: one indirect DMA per 128-row
  chunk (TRN2 SWDGE honors exactly one dynamic offset per output
  partition row - multi-offset APs replicate idx[p,0], probe-verified;
  the batched InstDMAGatherAnt path wedges this terminal's cores when
  self-triggered, and under the tile-managed PREPARE_ONLY+trigger_dma
  flow MultiCoreSim flags consumers racing the deferred DMA, so neither
  form is safe here - both probe-verified), scatter-add via one-hot
  stationary matmuls with fp32
  PSUM accumulation, self-loops via identity-stationary matmuls from
  the SBUF-resident fp8 shard, relu + symmetric-norm post-scale fused
  into the PSUM eviction. One-hots are streamed from DRAM per
  GRP-chunk group (too large for SBUF residency).
"""

import sys

sys.path.insert(0, "/opt/trn_rl_repo")

import numpy as np
import ml_dtypes

import concourse.bacc as bacc
import concourse.bass as bass
import concourse.mybir as mybir
from concourse import tile
from concourse.bass_utils import run_bass_kernel_spmd

BF16 = ml_dtypes.bfloat16
F8E3 = ml_dtypes.float8_e3m4
SCALES = (8.0, 16.0, 64.0)  # per-layer fp8 table scales (range-validated on inputs)

N = 10000
F = 512
NCORES = 8
P = N // NCORES          # 1250 nodes per core
NW = 10                  # dest windows per core (128 rows each)
PW = NW * 128            # 1280 padded nodes per core
HR = 640                 # rows per table half (windows 0-4 / 5-9)
PACK = 2                 # nodes packed per gathered table row
NPAIR = 5                # window pairs (2w, 2w+1) sharing one gather set
SG = ((0, 1), (2, 3), (4,))  # supergroups of pairs (PSUM: 2 pairs = 4 banks)
GRP = 8                  # chunks per gather/one-hot staging tile


def _preprocess(x, edge_index, Ws, bs):
    """Host-side: graph normalization + per-core gather/scatter plans."""
    ei = np.asarray(edge_index).astype(np.int64)
    deg = np.ones(N, np.float32)
    np.add.at(deg, ei[1], 1.0)
    dis = np.where(deg > 0, 1.0 / np.sqrt(np.maximum(deg, 1.0)), 0.0).astype(
        np.float32
    )
    row, col = ei[0], ei[1]

    # bucket edges by (core, window-pair, source-half); keep per-edge info
    edges = [[[None, None] for _ in range(NPAIR)] for _ in range(NCORES)]
    core_of = col // P
    wloc = (col - core_of * P) // 128
    half_of = (row % P) // HR  # 0 for local rows 0..639, 1 for 640..1249
    for c in range(NCORES):
        mc = core_of == c
        rc, cc, hc = row[mc], col[mc] - c * P, half_of[mc]
        wc = cc // 128
        for pi in range(NPAIR):
            for h in range(2):
                m = (wc // 2 == pi) & (hc == h)
                edges[c][pi][h] = (rc[m], cc[m] % 128, wc[m] % 2)

    def packrows(c, pi, h):
        s = edges[c][pi][h][0]
        lam = (s % P) - h * HR
        return (s // P) * (HR // PACK) + lam // PACK, (s % P) % PACK

    # uniform chunk counts over deduped PACKed rows (SPMD: one program)
    nch = [
        [
            max(
                1,
                max(
                    (len(np.unique(packrows(c, pi, h)[0])) + 127) // 128
                    for c in range(NCORES)
                ),
            )
            for h in range(2)
        ]
        for pi in range(NPAIR)
    ]

    # chunk processing order: per supergroup: all half-0 chunks (pair
    # ascending), then per-pair half-1 chunks. chunk_map rows:
    #   (pair, is_first_of_pair, is_last_of_pair)
    chunk_map = []
    chunk_halves = []
    order = []  # (pi, h, k)
    for sg in SG:
        for pi in sg:
            for k in range(nch[pi][0]):
                order.append((pi, 0, k))
                chunk_map.append((pi, k == 0, False))
                chunk_halves.append(0)
        for pi in sg:
            for k in range(nch[pi][1]):
                order.append((pi, 1, k))
                chunk_map.append((pi, False, k == nch[pi][1] - 1))
                chunk_halves.append(1)
    nchunk = len(order)
    chunk_base = {}
    b = 0
    for (pi, h, k) in order:
        chunk_base[(pi, h, k)] = b
        b += 1

    per_core = []
    for c in range(NCORES):
        idx32 = np.zeros((128, nchunk), np.int32)
        onehot = np.zeros((128, nchunk * 2 * PACK * 128), np.float32)
        for pi in range(NPAIR):
            for h in range(2):
                _, d, v = edges[c][pi][h]
                prow, lane = packrows(c, pi, h)
                uniqp, invp = np.unique(prow, return_inverse=True)
                for k in range(nch[pi][h]):
                    t = chunk_base[(pi, h, k)]
                    seg = uniqp[k * 128 : (k + 1) * 128]
                    idx32[: len(seg), t] = seg
                # edge e: pack slot sigma, lane l, window parity v, dest d:
                # one-hot block = ((chunk*2 + v)*PACK + l)
                sigma_e = invp
                t0 = chunk_base[(pi, h, 0)]
                blk = ((t0 + sigma_e // 128) * 2 + v) * PACK + lane
                np.add.at(
                    onehot,
                    (sigma_e % 128, blk * 128 + d),
                    np.float32(1.0),
                )
        onehot = onehot.astype(F8E3)

        dis_sc = np.zeros((128, 6 * NW), np.float32)
        nloc = np.arange(P)
        dv = dis[c * P : (c + 1) * P]
        for l in range(3):
            dis_sc[nloc % 128, l * NW + nloc // 128] = SCALES[l] * dv
            dis_sc[nloc % 128, (3 + l) * NW + nloc // 128] = dv / SCALES[l]

        xT = np.zeros((F, PW), BF16)
        xT[:, :P] = np.asarray(x)[c * P : (c + 1) * P].T.astype(BF16)
        per_core.append(
            {"xt": xT, "dis": dis_sc, "gidx": idx32, "onehot": onehot}
        )

    wall = np.stack([np.asarray(w).astype(BF16) for w in Ws])
    ident = np.eye(128, dtype=BF16)
    ident8 = np.eye(128, dtype=F8E3)
    has_bias = any(np.any(np.asarray(b)) for b in bs)
    shared = {"wall": wall, "ident": ident, "ident8": ident8}
    if has_bias:
        brep = np.stack([np.asarray(b).astype(BF16) for b in bs])
        shared["brep"] = brep.reshape(1, 3 * F)
        for c in range(NCORES):
            iv = np.zeros((1, PW), BF16)
            iv[0, :P] = (1.0 / dis[c * P : (c + 1) * P]).astype(BF16)
            per_core[c]["invdis"] = iv
    meta = {
        "nchunk": nchunk,
        "chunk_map": tuple(chunk_map),
        "chunk_halves": tuple(chunk_halves),
        "has_bias": has_bias,
    }
    return per_core, shared, meta


def _build(meta, mock_cc=False, repeat=1):
    nchunk = meta["nchunk"]
    chunk_map, has_bias = meta["chunk_map"], meta["has_bias"]
    chunk_halves = meta["chunk_halves"]
    bf = mybir.dt.bfloat16
    f8 = mybir.dt.float8e3
    f32 = mybir.dt.float32

    nc = bacc.Bacc(
        "TRN2",
        target_bir_lowering=False,
        debug=False,
        num_devices=1 if mock_cc else NCORES,
        dynamic_dma_scratch_size=65536,
    )
    xt_d = nc.dram_tensor("xt", [F, PW], bf, kind="ExternalInput")
    wall_d = nc.dram_tensor("wall", [3, F, F], bf, kind="ExternalInput")
    dis_d = nc.dram_tensor("dis", [128, 6 * NW], f32, kind="ExternalInput")
    gidx_d = nc.dram_tensor(
        "gidx", [128, nchunk], mybir.dt.int32, kind="ExternalInput"
    )
    oh_d = nc.dram_tensor(
        "onehot", [128, nchunk * 2 * PACK * 128], f8, kind="ExternalInput"
    )
    id_d = nc.dram_tensor("ident", [128, 128], bf, kind="ExternalInput")
    id8_d = nc.dram_tensor("ident8", [128, 128], f8, kind="ExternalInput")
    if has_bias:
        brep_d = nc.dram_tensor("brep", [1, 3 * F], bf, kind="ExternalInput")
        invdis_d = nc.dram_tensor("invdis", [1, PW], bf, kind="ExternalInput")
    y_d = nc.dram_tensor("y", [PW, F], f32, kind="ExternalOutput")

    with tile.TileContext(nc) as tc:
        with (
            tc.tile_pool(name="const", bufs=1) as cp,
            tc.tile_pool(name="work", bufs=1) as wp,
            tc.tile_pool(name="gatp", bufs=2) as gp,
            tc.tile_pool(name="ohp", bufs=2) as op,
            tc.tile_pool(name="psum", bufs=1, space="PSUM") as pp,
            tc.tile_pool(name="ptr", bufs=1, space="PSUM") as ppt,
            tc.tile_pool(name="ccin_p", bufs=2, space="DRAM") as dp_in,
            tc.tile_pool(name="ccout_p", bufs=2, space="DRAM") as dp_out,
            tc.tile_pool(name="tok_p", bufs=1, space="DRAM") as dp_tok,
        ):
            # constants
            w_t = cp.tile([128, 3 * 4 * F], bf, name="w_t")
            for l in range(3):
                for kc in range(4):
                    nc.sync.dma_start(
                        w_t[:, (l * 4 + kc) * F : (l * 4 + kc + 1) * F],
                        wall_d[l, kc * 128 : (kc + 1) * 128, :],
                    )
            dis_t = cp.tile([128, 6 * NW], f32, name="dis_t")
            nc.sync.dma_start(dis_t[:], dis_d[:])
            idx_t = cp.tile([128, nchunk], mybir.dt.int32, name="idx_t")
            nc.sync.dma_start(idx_t[:], gidx_d[:])
            id_t = cp.tile([128, 128], bf, name="id_t")
            nc.sync.dma_start(id_t[:], id_d[:])
            id8_t = cp.tile([128, 128], f8, name="id8_t")
            nc.sync.dma_start(id8_t[:], id8_d[:])
            if has_bias:
                brep_t = cp.tile([1, 3 * F], bf, name="brep_t")
                nc.sync.dma_start(brep_t[:], brep_d[:])
                invdis_t = cp.tile([1, PW], bf, name="invdis_t")
                nc.sync.dma_start(invdis_t[:], invdis_d[:])

            def allgather(cc_in, half, l, rep):
                """AG of cc_in rows [half*HR:(half+1)*HR) of every rank into a
                [NCORES*HR, F] table (per-rank stripes of HR rows)."""
                cc_out = dp_out.tile(
                    [NCORES * HR, F],
                    f8,
                    tag=f"ccoh{half}",
                    addr_space="Local" if mock_cc else "Shared",
                    name=f"ccout{l}_{half}_{rep}",
                )
                src = cc_in[half * HR : (half + 1) * HR, :]
                if mock_cc:
                    for r in range(NCORES):
                        nc.sync.dma_start(cc_out[r * HR : (r + 1) * HR, :], src)
                else:
                    nc.gpsimd.collective_compute(
                        "AllGather",
                        mybir.AluOpType.bypass,
                        replica_groups=[list(range(NCORES))],
                        ins=[src],
                        outs=[cc_out[:]],
                    )
                return cc_out

            def gemm_window(l, w, zt_src, h_t, cc_in, rep):
                """H'[w] = dis * (Z[w] @ W_l) -> h_t[:, w] (bf16) -> cc_in."""
                pg = pp.tile(
                    [128, F], f32, tag="gemm", bufs=2, name=f"pg{l}_{w}_{rep}"
                )
                for kc in range(4):
                    nc.tensor.matmul(
                        pg[:],
                        zt_src[:, kc * PW + w * 128 : kc * PW + (w + 1) * 128],
                        w_t[:, (l * 4 + kc) * F : (l * 4 + kc + 1) * F],
                        start=(kc == 0),
                        stop=(kc == 3),
                    )
                nc.scalar.activation(
                    h_t[:, w * F : (w + 1) * F],
                    pg[:],
                    mybir.ActivationFunctionType.Copy,
                    scale=dis_t[:, l * NW + w : l * NW + w + 1],
                )
                nc.sync.dma_start(
                    cc_in[w * 128 : (w + 1) * 128, :],
                    h_t[:, w * F : (w + 1) * F],
                )

            tok_d = None
            for rep in range(repeat):
                # ---- prologue: load Z^T, GEMM layer 0, fire AGs ----
                zt = wp.tile([128, 4 * PW], bf, tag="ZT", name=f"zt0_{rep}")
                for kc in range(4):
                    nc.sync.dma_start(
                        zt[:, kc * PW : (kc + 1) * PW],
                        xt_d[kc * 128 : (kc + 1) * 128, :],
                    )
                if rep > 0:
                    # timing builds: chain rep on previous rep's output so the
                    # marginal-slope measurement reflects single-shot latency
                    # (corrupts 16 staged Z values; repeat>1 is never used for
                    # the graded numerics)
                    tok_s = wp.tile([1, 16], f32, tag="toks", name=f"toks{rep}")
                    nc.sync.dma_start(tok_s[:], tok_d[:])
                    nc.scalar.activation(
                        zt[0:1, 0:16],
                        tok_s[:],
                        mybir.ActivationFunctionType.Copy,
                    )
                h_t = wp.tile([128, NW * F], f8, tag="H", bufs=2, name=f"h0_{rep}")
                cc_in = dp_in.tile([PW, F], f8, tag="ccin", name=f"ccin0_{rep}")
                cc_out_h = [None, None]
                for w in range(NW):
                    gemm_window(0, w, zt, h_t, cc_in, rep)
                    if w == 4:
                        cc_out_h[0] = allgather(cc_in, 0, 0, rep)
                cc_out_h[1] = allgather(cc_in, 1, 0, rep)

                # ---- 3 scatter phases ----
                for l in range(3):
                    z_t = (
                        wp.tile([128, NW * F], bf, tag="Z", name=f"z{l}_{rep}")
                        if l < 2
                        else None
                    )
                    ztn = (
                        wp.tile([128, 4 * PW], bf, tag="ZT", name=f"zt{l + 1}_{rep}")
                        if l < 2
                        else None
                    )
                    h_next = (
                        wp.tile(
                            [128, NW * F], f8, tag="H", bufs=2, name=f"h{l + 1}_{rep}"
                        )
                        if l < 2
                        else None
                    )
                    cc_in_next = (
                        dp_in.tile([PW, F], f8, tag="ccin", name=f"ccin{l + 1}_{rep}")
                        if l < 2
                        else None
                    )
                    cc_next = [None, None]

                    ps_w = {}
                    gat = None
                    ohg = None
                    for t in range(nchunk):
                        if t % GRP == 0:
                            gat = gp.tile(
                                [128, GRP * PACK * F],
                                f8,
                                tag="gat",
                                name=f"gat{l}_{t // GRP}_{rep}",
                            )
                            ohg = op.tile(
                                [128, GRP * 2 * PACK * 128],
                                f8,
                                tag="oh",
                                name=f"oh{l}_{t // GRP}_{rep}",
                            )
                            ghi = min(t + GRP, nchunk)
                            nc.sync.dma_start(
                                ohg[:, : (ghi - t) * 2 * PACK * 128],
                                oh_d[:, t * 2 * PACK * 128 : ghi * 2 * PACK * 128],
                            )
                        c = t % GRP
                        pi, first, last = chunk_map[t]
                        nc.gpsimd.indirect_dma_start(
                            gat[:, c * PACK * F : (c + 1) * PACK * F],
                            None,
                            cc_out_h[chunk_halves[t]][:].rearrange(
                                "(r q) f -> r (q f)", q=PACK
                            ),
                            bass.IndirectOffsetOnAxis(
                                ap=idx_t[:, t : t + 1], axis=0
                            ),
                        )
                        if first:
                            for v in range(2):
                                w = 2 * pi + v
                                ps = pp.tile(
                                    [128, F],
                                    f32,
                                    tag=f"scat{w % 4}",
                                    name=f"ps{l}_{w}_{rep}",
                                )
                                ps_w[w] = ps
                                # self-loop: psum[d] += H'[d]
                                nc.tensor.matmul(
                                    ps[:],
                                    id8_t[:],
                                    h_t[:, w * F : (w + 1) * F],
                                    start=True,
                                    stop=False,
                                )
                        for v in range(2):
                            ps = ps_w[2 * pi + v]
                            for ln in range(PACK):
                                nc.tensor.matmul(
                                    ps[:],
                                    ohg[
                                        :,
                                        ((c * 2 + v) * PACK + ln)
                                        * 128 : ((c * 2 + v) * PACK + ln + 1)
                                        * 128,
                                    ],
                                    gat[
                                        :,
                                        (c * PACK + ln) * F : (c * PACK + ln + 1) * F,
                                    ],
                                    start=False,
                                    stop=(
                                        last and ln == PACK - 1 and not has_bias
                                    ),
                                )
                        if last:
                          for v in range(2):
                            w = 2 * pi + v
                            ps = ps_w[w]
                            if has_bias:
                                nc.tensor.matmul(
                                    ps[:],
                                    invdis_t[0:1, w * 128 : (w + 1) * 128],
                                    brep_t[0:1, l * F : (l + 1) * F],
                                    start=False,
                                    stop=True,
                                )
                            if l < 2:
                                nc.scalar.activation(
                                    z_t[:, w * F : (w + 1) * F],
                                    ps[:],
                                    mybir.ActivationFunctionType.Relu,
                                    scale=dis_t[:, (3 + l) * NW + w : (3 + l) * NW + w + 1],
                                )
                                for fc in range(4):
                                    pt = ppt.tile(
                                        [128, 128],
                                        bf,
                                        tag="tr",
                                        name=f"pt{l}_{w}_{fc}_{rep}",
                                    )
                                    nc.tensor.transpose(
                                        pt[:],
                                        z_t[
                                            :,
                                            w * F + fc * 128 : w * F + (fc + 1) * 128,
                                        ],
                                        id_t[:],
                                    )
                                    nc.vector.tensor_copy(
                                        ztn[
                                            :,
                                            fc * PW + w * 128 : fc * PW + (w + 1) * 128,
                                        ],
                                        pt[:],
                                    )
                                gemm_window(l + 1, w, ztn, h_next, cc_in_next, rep)
                                if w == 4:
                                    cc_next[0] = allgather(cc_in_next, 0, l + 1, rep)
                                elif w == 9:
                                    cc_next[1] = allgather(cc_in_next, 1, l + 1, rep)
                            else:
                                yw = wp.tile(
                                    [128, F], f32, tag="Y", name=f"yw{w}_{rep}"
                                )
                                nc.scalar.activation(
                                    yw[:],
                                    ps[:],
                                    mybir.ActivationFunctionType.Copy,
                                    scale=dis_t[:, (3 + l) * NW + w : (3 + l) * NW + w + 1],
                                )
                                nc.sync.dma_start(
                                    y_d[w * 128 : (w + 1) * 128, :], yw[:]
                                )
                                if w == 9 and repeat > 1:
                                    tok_d = dp_tok.tile(
                                        [1, 16], f32, tag="tok", name=f"tok{rep}"
                                    )
                                    nc.sync.dma_start(tok_d[:], yw[0:1, 0:16])
                    if l < 2:
                        zt = ztn
                        h_t = h_next
                        cc_in = cc_in_next
                        cc_out_h = cc_next

    nc.compile()
    return nc


_CACHE = {}


def _get_program(meta):
    key = (meta["nchunk"], meta["chunk_map"], meta["has_bias"], meta["chunk_halves"])
    if key not in _CACHE:
        _CACHE[key] = _build(meta)
    return _CACHE[key]


def kernel(x, edge_index, W1, b1, W2, b2, W3, b3):
    per_core, shared, meta = _preprocess(
        x, edge_index, [W1, W2, W3], [b1, b2, b3]
    )
    nc = _get_program(meta)
    in_maps = [dict(pc, **shared) for pc in per_core]
    res = run_bass_kernel_spmd(nc, in_maps, list(range(NCORES)))
    out = np.concatenate(
        [res.results[c]["y"][:P] for c in range(NCORES)], axis=0
    )
    return out.astype(np.float32)
